# revision 1
# baseline (speedup 1.0000x reference)
"""Distributed sparse embedding lookup (mean combiner) on 8 Trainium2 cores.

Strategy (data-parallel over output rows, table replicated on every core):
  - Each core owns 1/8 of the output rows (13312 = 104*128). row_indices is
    sorted, so each core's keys are a contiguous slice of the input.
  - Keys are bucketed into 31 vocab windows of 32768 rows (dma_gather index
    tensors are int16). Within a window, keys are split into column-aligned
    chunks such that NO chunk contains two keys of the same output row
    (dma_scatter_add loses updates on duplicate targets within one
    instruction - HW-verified), distributing each row's in-window keys
    round-robin over the window's chunks.
  - Device pipeline per window: dma_gather (random 256B table rows, HBM ->
    SBUF) -> DVE multiply by per-key 1/count (mean pre-scaling, 0-stride
    broadcast along the 64-dim) -> per chunk one dma_scatter_add in
    SBUF-destination parity mode into one of two accumulator pairs
    (alternating, so the WAW serialization chains halve). Accumulator
    layout: output row r -> partition r%128, slot r//128; even slots in
    acc_a*, odd slots in acc_b* of the pair.
  - Final merge: pair0 + pair1 per parity on DVE, then two strided dense
    DMAs into the [13312, 64] output. Host concatenates the 8 core outputs.

All index preprocessing is host-side numpy; all table-data movement and
floating-point arithmetic run on the device.
"""
import numpy as np

_B, _S, _D = 4096, 26, 64
_V = 1_000_000
_M = 8
_R = _B * _S            # 106496 output rows
_RC = _R // _M          # 13312 rows per core = 104 slots * 128
_WIN = 32768
_NWIN = (_V + _WIN - 1) // _WIN      # 31
_ORC = _RC + 128        # +128 pad rows; pads scatter-add into row _RC
_NSLOT = _ORC // 128                 # 105 slots (even: 53, odd: 52)
_BG = 1024              # max num_idxs per dma_gather (HW ring validated)
_BS = 768               # max num_idxs per dma_scatter_add (HW-validated)
_NPAIR = 4              # accumulator pairs (independent WAW chains)

_prog_cache = {}


def _cdiv(a, b):
    return (a + b - 1) // b


def _pack16(v, budget, pad):
    out = np.full(budget, pad, dtype=v.dtype)
    out[: len(v)] = v
    return np.tile(out.reshape(-1, 16).T, (8, 1))


def _pack128(v, budget, pad):
    out = np.full(budget, pad, dtype=v.dtype)
    out[: len(v)] = v
    return out.reshape(-1, 128).T


def _chunk_window(keys, rows, invc, n_chunks, cap):
    """Distribute one window's keys into n_chunks lists, no row repeated
    within a chunk and no chunk above cap. keys are row-major; same-row keys
    are adjacent. Returns None if infeasible with this n_chunks."""
    out_k = [[] for _ in range(n_chunks)]
    out_r = [[] for _ in range(n_chunks)]
    out_i = [[] for _ in range(n_chunks)]
    fill = [0] * n_chunks
    n = len(keys)
    i = 0
    nxt = 0
    while i < n:
        j = i
        r = rows[i]
        while j < n and rows[j] == r:
            j += 1
        used = []
        for t in range(i, j):
            c = None
            for probe in range(n_chunks):
                cand = (nxt + t - i + probe) % n_chunks
                if fill[cand] < cap and cand not in used:
                    c = cand
                    break
            if c is None:
                return None
            used.append(c)
            out_k[c].append(keys[t])
            out_r[c].append(r)
            out_i[c].append(invc[t])
            fill[c] += 1
        nxt = (nxt + 1) % n_chunks
        i = j
    return out_k, out_r, out_i


def _prep(values, row_indices):
    """Returns (gather_budgets, chunk_budgets, in_maps)."""
    values = np.asarray(values).astype(np.int64)
    row_indices = np.asarray(row_indices).astype(np.int64)
    if np.any(np.diff(row_indices) < 0):
        order = np.argsort(row_indices, kind="stable")
        values, row_indices = values[order], row_indices[order]
    bounds = np.searchsorted(row_indices, np.arange(_M + 1) * _RC)
    per_core = []       # per core: per window: (keys, rows, invc)
    for c in range(_M):
        lo, hi = bounds[c], bounds[c + 1]
        keys = values[lo:hi]
        rows = row_indices[lo:hi] - c * _RC
        counts = np.bincount(rows, minlength=_RC).astype(np.float32)
        invc = (1.0 / np.maximum(counts, 1.0))[rows].astype(np.float32)
        # sort by (window, row): row-major within each window
        w = keys // _WIN
        order = np.lexsort((rows, w))
        ks, rs, iv = keys[order], rows[order], invc[order]
        wb = np.searchsorted(ks // _WIN, np.arange(_NWIN + 1))
        wins = []
        for wi in range(_NWIN):
            sl = slice(wb[wi], wb[wi + 1])
            wins.append((ks[sl] - wi * _WIN, rs[sl], iv[sl]))
        per_core.append(wins)

    # per window: number of chunks (same for all cores)
    n_chunks_w = []
    for wi in range(_NWIN):
        need = 1
        for c in range(_M):
            k, r, iv = per_core[c][wi]
            need = max(need, _cdiv(len(k), _BS))
            if len(r):
                _un, cnt = np.unique(r, return_counts=True)
                need = max(need, int(cnt.max()))
        n_chunks_w.append(need)

    # distribute into chunks; chunk budgets = max fill over cores, x128.
    # Raise n_chunks until every core fits the per-instruction cap.
    per_core_chunks = [[None] * _NWIN for _ in range(_M)]
    for wi in range(_NWIN):
        while True:
            ok = True
            for c in range(_M):
                k, r, iv = per_core[c][wi]
                res = _chunk_window(k, r, iv, n_chunks_w[wi], _BS)
                if res is None:
                    ok = False
                    break
                per_core_chunks[c][wi] = res
            if ok:
                break
            n_chunks_w[wi] += 1
    chunk_budgets = []   # flat list over (window, chunk)
    for wi in range(_NWIN):
        for ci in range(n_chunks_w[wi]):
            mx = max(len(per_core_chunks[c][wi][0][ci]) for c in range(_M))
            chunk_budgets.append((wi, max(_cdiv(mx, 128), 1) * 128))

    in_maps = []
    for c in range(_M):
        g_parts, s_parts, i_parts = [], [], []
        ptr = {wi: 0 for wi in range(_NWIN)}
        for wi, bud in chunk_budgets:
            ci = ptr[wi]
            ptr[wi] += 1
            ck, cr, ci_v = per_core_chunks[c][wi]
            k = np.asarray(ck[ci], np.int16)
            r = np.asarray(cr[ci], np.int16)
            iv = np.asarray(ci_v[ci], np.float32)
            g_parts.append(_pack16(k, bud, np.int16(0)))
            s_parts.append(_pack16(r, bud, np.int16(_RC)))  # pad -> dedicated pad slot
            i_parts.append(_pack128(iv, bud, np.float32(0.0)))   # zero contribution
        in_maps.append({
            "gidx": np.ascontiguousarray(np.concatenate(g_parts, axis=1)),
            "sidx": np.ascontiguousarray(np.concatenate(s_parts, axis=1)),
            "invc": np.ascontiguousarray(np.concatenate(i_parts, axis=1)),
        })
    return chunk_budgets, in_maps


def _build(chunk_budgets, n_reps=1):
    from concourse import bacc, mybir, tile

    nc = bacc.Bacc(None, target_bir_lowering=False, debug=False,
                   num_swdge_queues=1)
    table = nc.dram_tensor("table", [_V, _D], mybir.dt.float32,
                           kind="ExternalInput")
    gtot = sum(b // 16 for _w, b in chunk_budgets)
    ntot = sum(b // 128 for _w, b in chunk_budgets)
    gidx = nc.dram_tensor("gidx", [128, gtot], mybir.dt.int16,
                          kind="ExternalInput")
    sidx = nc.dram_tensor("sidx", [128, gtot], mybir.dt.int16,
                          kind="ExternalInput")
    invc = nc.dram_tensor("invc", [128, ntot], mybir.dt.float32,
                          kind="ExternalInput")
    out = nc.dram_tensor("out", [_ORC, _D], mybir.dt.float32,
                         kind="ExternalOutput")
    HGA = (_NSLOT + 1) // 2   # even-slot groups (incl. pad slot)
    HGB = _NSLOT // 2         # odd-slot groups

    with tile.TileContext(nc) as tc:
        with (
            tc.tile_pool(name="acc", bufs=1) as apool,
            tc.tile_pool(name="data", bufs=6) as dpool,
            tc.tile_pool(name="meta", bufs=1) as mpool,
        ):
            accs = []
            for p in range(_NPAIR):
                aa = apool.tile([128, HGA, _D], mybir.dt.float32, tag=f"aa{p}")
                ab = apool.tile([128, HGA, _D], mybir.dt.float32, tag=f"ab{p}")
                nc.vector.memset(aa[:], 0.0)
                nc.vector.memset(ab[:], 0.0)
                accs.append((aa, ab))

            # group consecutive same-window chunks into one gather of <= _BG
            ggroups = []
            for wi, bud in chunk_budgets:
                if (ggroups and ggroups[-1][0] == wi
                        and ggroups[-1][1] + bud <= _BG):
                    ggroups[-1][1] += bud
                    ggroups[-1][2].append(bud)
                else:
                    ggroups.append([wi, bud, [bud]])

            # preload all index/scale metadata once; slice on-chip
            gix = mpool.tile([128, gtot], mybir.dt.int16, tag="gix")
            six = mpool.tile([128, gtot], mybir.dt.int16, tag="six")
            ivx = mpool.tile([128, ntot], mybir.dt.float32, tag="ivx")
            nc.sync.dma_start(out=gix[:], in_=gidx[:])
            nc.sync.dma_start(out=six[:], in_=sidx[:])
            nc.sync.dma_start(out=ivx[:], in_=invc[:])

            for _rep in range(n_reps):
                goff = noff = 0
                chain = 0
                for wi, total, buds in ggroups:
                    nt = total // 128
                    base = wi * _WIN
                    wsize = min(_WIN, _V - base)
                    gat = dpool.tile([128, nt, _D], mybir.dt.float32, tag="gat")
                    nc.gpsimd.dma_gather(
                        out_ap=gat[:], in_ap=table[base:base + wsize, :],
                        idxs_ap=gix[:, goff:goff + total // 16],
                        num_idxs=total, num_idxs_reg=total,
                        elem_size=_D, queue_num=0,
                    )
                    sc = dpool.tile([128, nt, _D], mybir.dt.float32, tag="sc")
                    nc.vector.tensor_tensor(
                        out=sc[:], in0=gat[:],
                        in1=ivx[:, noff:noff + nt, None].to_broadcast(
                            [128, nt, _D]),
                        op=mybir.AluOpType.mult,
                    )
                    coff = 0
                    for bud in buds:
                        aa, ab = accs[chain % _NPAIR]
                        chain += 1
                        nc.gpsimd.dma_scatter_add(
                            out_ap=aa[:], in_ap=sc[:, coff:coff + bud // 128, :],
                            idxs_ap=six[:, goff + coff * 8:
                                        goff + coff * 8 + bud // 16],
                            num_idxs=bud, num_idxs_reg=bud,
                            elem_size=_D, queue_num=0, sbuf_tokens_per_rank=128,
                            parity_reg=0, out_ap_other=ab[:],
                        )
                        coff += bud // 128
                    goff += total // 16
                    noff += nt

            # merge pairs in place into accs[0] and write out
            for par in range(2):
                hg = HGA if par == 0 else HGB
                acc0 = accs[0][par][:, :hg, :]
                for p in range(1, _NPAIR):
                    nc.vector.tensor_add(out=acc0, in0=acc0,
                                         in1=accs[p][par][:, :hg, :])
                out_view = out[:].rearrange("(s p) d -> p s d", p=128)
                nc.sync.dma_start(out=out_view[:, par::2, :], in_=acc0)
    nc.compile()
    return nc


def _state(values, row_indices, emb_table, n_reps=1):
    chunk_budgets, in_maps = _prep(values, row_indices)
    key = (tuple(chunk_budgets), n_reps)
    if key not in _prog_cache:
        _prog_cache[key] = _build(chunk_budgets, n_reps=n_reps)
    nc = _prog_cache[key]
    table = np.ascontiguousarray(np.asarray(emb_table, dtype=np.float32))
    for m in in_maps:
        m["table"] = table
    return nc, in_maps


def kernel(values, row_indices, emb_table):
    from concourse.bass_utils import run_bass_kernel_spmd

    nc, in_maps = _state(values, row_indices, emb_table)
    res = run_bass_kernel_spmd(nc, in_maps, core_ids=list(range(_M)))
    full = np.concatenate(
        [np.asarray(res.results[c]["out"])[:_RC] for c in range(_M)], axis=0)
    return np.ascontiguousarray(full.reshape(_B, _S, _D))



# revision 9
# speedup vs baseline: 5.3346x; 5.3346x over previous
"""Distributed sparse embedding lookup (mean combiner) on 8 Trainium2 cores.

Architecture (data-parallel over output rows, table replicated per core,
two-phase gather through an HBM staging buffer, matmul combine):

  - Each core owns 1/8 of the output rows (13312 = 208 blocks x 64 rows).
    row_indices is sorted, so each core's keys are a contiguous input slice.
  - Output slots: block b gets tiles_pb[b] = max-over-cores ceil(keys_b/128)
    tiles of 128 gather slots (row-major within the block, pad slots at the
    tile tails). Slots are grouped into SUPERBLOCKS of 28672 so that each
    superblock's staging region fits one int16 dma_gather window.
  - Phase A: per (superblock, table-window of 32768 vocab rows), dma_gather
    the slots' unique keys (f32 256B rows, <=1024 idxs per instruction — the
    HW-validated ring limit) and stream the result to an HBM staging buffer
    with dense HWDGE writes alternating between the SP and ACT engines.
    Staging order = (superblock, window, key) — affine from the gather.
  - Phase B: per slot chunk, dma_gather row-major from this superblock's
    staging region (idx = position of the slot's key inside the region),
    cast f32->bf16 on DVE, then one PE matmul per 128-slot tile:
    lhsT = gathered [128 keys, 64 emb] bf16, rhs = host-built one-hot
    A^T [128 keys, 64 rows-in-block] bf16, accumulated into a PSUM bank of
    512 rows (first tile of each block starts the window, so no PSUM
    zeroing). 1/count is applied per row during the PSUM->SBUF copy on DVE.
  - Staging is a DRAM tile pool with bufs=2 so reps pipeline.
  - Final: out.T SBUF bf16 -> DRAM f32 cast-DMA once; host transposes and
    concatenates the 8 core slabs.

All tile/budget bookkeeping is shared across cores (max over cores), so one
SPMD program serves all 8; per-core variation lives in uploaded tensors.
"""
import numpy as np
import ml_dtypes

_B, _S, _D = 4096, 26, 64
_V = 1_000_000
_M = 8
_R = _B * _S              # 106496 output rows
_RC = _R // _M            # 13312 rows per core
_BLK = 64                 # rows per block = one-hot width
_BANK = 512               # rows per PSUM bank group
_NBLK = _RC // _BLK       # 208
_NGRP = _RC // _BANK      # 26
_BPG = _BANK // _BLK      # 8 blocks per group
_WIN = 32768              # table window (int16 idx)
_NWIN = (_V + _WIN - 1) // _WIN
_SBS = 28672              # slots per superblock (28 * 1024)
_CH = 1024                # keys per dma_gather instruction
_BF16 = ml_dtypes.bfloat16

_prog_cache = {}


def _cdiv(a, b):
    return (a + b - 1) // b


def _pack16(v, budget, pad):
    out = np.full(budget, pad, dtype=np.int16)
    out[: len(v)] = v
    return np.tile(out.reshape(-1, 16).T, (8, 1))


def _prep(values, row_indices):
    values = np.asarray(values).astype(np.int64)
    row_indices = np.asarray(row_indices).astype(np.int64)
    if np.any(np.diff(row_indices) < 0):
        order = np.argsort(row_indices, kind="stable")
        values, row_indices = values[order], row_indices[order]
    bounds = np.searchsorted(row_indices, np.arange(_M + 1) * _RC)

    cores = []
    kcb = np.zeros((_M, _NBLK), dtype=np.int64)
    for c in range(_M):
        lo, hi = bounds[c], bounds[c + 1]
        keys = values[lo:hi]
        rows = row_indices[lo:hi] - c * _RC
        kcb[c] = np.bincount(rows // _BLK, minlength=_NBLK)
        cores.append((keys, rows))

    tiles_pb = np.maximum(1, -(-kcb.max(axis=0) // 128))
    col0 = np.concatenate([[0], np.cumsum(tiles_pb)])
    T = int(col0[-1])
    nslot = T * 128
    nsb = _cdiv(nslot, _SBS)

    # per core: slot -> (key or -1); slot superblock id
    slot_key = np.full((_M, nslot), -1, dtype=np.int64)
    slot_row = np.full((_M, nslot), 0, dtype=np.int64)
    for c in range(_M):
        keys, rows = cores[c]
        n = len(keys)
        blk = rows // _BLK
        bstart = np.searchsorted(blk, np.arange(_NBLK))
        islot = (col0[blk] * 128 + (np.arange(n) - bstart[blk]))
        slot_key[c, islot] = keys
        slot_row[c, islot] = rows

    slot_sb = np.arange(nslot) // _SBS
    slot_win = np.where(slot_key >= 0, slot_key // _WIN, -1)   # [M, nslot]

    # phase-A budgets per (superblock, window): unique keys, max over cores,
    # rounded to 128
    uniq = {}      # (c, s, w) -> (unique_keys_ascending, slot_positions_map)
    bud = np.zeros((nsb, _NWIN), dtype=np.int64)
    for c in range(_M):
        for s in range(nsb):
            m_s = slot_sb == s
            for w in range(_NWIN):
                m = m_s & (slot_win[c] == w)
                u = np.unique(slot_key[c][m])
                uniq[(c, s, w)] = u
                bud[s, w] = max(bud[s, w], len(u))
    bud = -(-bud // 128) * 128
    sb_size = bud.sum(axis=1)              # staging rows per superblock
    assert sb_size.max() <= _WIN, sb_size
    sb_base = np.concatenate([[0], np.cumsum(sb_size)])
    spad = int(sb_base[-1])

    # phase-A chunk list (shared): (s, w, staging_pos, n_budget) split <=CH
    chunksA = []
    for s in range(nsb):
        pos = int(sb_base[s])
        for w in range(_NWIN):
            left = int(bud[s, w])
            while left > 0:
                n = min(_CH, left)
                chunksA.append((s, w, pos, n))
                pos += n
                left -= n
    # phase-B chunk list: (s, slot_start, n_slots) split <=CH within sblocks
    chunksB = []
    for s in range(nsb):
        st = s * _SBS
        en = min(nslot, (s + 1) * _SBS)
        p = st
        while p < en:
            n = min(_CH, en - p)
            chunksB.append((s, p, n))
            p += n

    in_maps = []
    for c in range(_M):
        # staging position of each (s, w) unique key. Within each <=1024-key
        # gather chunk the staging order is TRANSPOSED (partition-major:
        # local key j -> (j%128)*(n/128) + j//128) so the HWDGE staging
        # write is one contiguous run per partition.
        posmap = {}
        gidx_parts = []
        wpos = {}
        for s in range(nsb):
            pos = int(sb_base[s])
            for w in range(_NWIN):
                u = uniq[(c, s, w)]
                wpos[(s, w)] = pos
                b_sw = int(bud[s, w])
                for i, k in enumerate(u):
                    ck = i // _CH
                    j = i % _CH
                    n_k = min(_CH, b_sw - ck * _CH)
                    posmap[(s, k)] = (pos + ck * _CH
                                      + (j % 128) * (n_k // 128) + j // 128)
                pos += b_sw
        # gidx stream per phase-A chunk (local window idx, pad=0)
        for (s, w, pos, n) in chunksA:
            u = uniq[(c, s, w)]
            off = pos - wpos[(s, w)]
            seg = (u[off:off + n] - w * _WIN).astype(np.int16)
            gidx_parts.append(_pack16(seg, n, np.int16(0)))
        gidx = np.concatenate(gidx_parts, axis=1)

        # ridx stream per phase-B chunk (position within superblock region)
        ridx_parts = []
        for (s, p, n) in chunksB:
            sk = slot_key[c, p:p + n]
            loc = np.zeros(n, dtype=np.int16)
            nzm = sk >= 0
            if nzm.any():
                loc[nzm] = np.array(
                    [posmap[(s, k)] for k in sk[nzm]],
                    dtype=np.int64) - sb_base[s]
            ridx_parts.append(_pack16(loc, n, np.int16(0)))
        ridx = np.concatenate(ridx_parts, axis=1)

        # A^T one-hot [128, T*BLK] bf16  (slot -> row-in-block)
        atab = np.zeros((128, T * _BLK), dtype=np.float32)
        sl = np.arange(nslot)
        colv = sl // 128
        pv = sl % 128
        realm = slot_key[c] >= 0
        blkv = np.zeros(nslot, dtype=np.int64)
        blkv[realm] = slot_row[c][realm] // _BLK
        atab[pv[realm], colv[realm] * _BLK
             + (slot_row[c][realm] - blkv[realm] * _BLK)] = 1.0

        keys, rows = cores[c]
        counts = np.bincount(rows, minlength=_RC).astype(np.float32)
        invc = 1.0 / np.maximum(counts, 1.0)

        in_maps.append({
            "gidx": np.ascontiguousarray(gidx),
            "ridx": np.ascontiguousarray(ridx),
            "atab": atab.astype(_BF16),
            "invc": np.broadcast_to(invc.astype(_BF16), (64, _RC)).copy(),
        })
    struct = (tuple(int(x) for x in tiles_pb), tuple(chunksA),
              tuple(chunksB), spad, tuple(int(x) for x in sb_base))
    return struct, in_maps


def _build(struct, n_reps=1):
    from concourse import bacc, mybir, tile

    tiles_pb, chunksA, chunksB, spad, sb_base = struct
    T = sum(tiles_pb)
    glen = sum(n for (_s, _w, _p, n) in chunksA)
    rlen = sum(n for (_s, _p, n) in chunksB)

    nc = bacc.Bacc(None, target_bir_lowering=False, debug=False,
                   num_swdge_queues=1)
    table = nc.dram_tensor("table", [_V, _D], mybir.dt.float32,
                           kind="ExternalInput")
    gidx_d = nc.dram_tensor("gidx", [128, glen // 16], mybir.dt.int16,
                            kind="ExternalInput")
    ridx_d = nc.dram_tensor("ridx", [128, rlen // 16], mybir.dt.int16,
                            kind="ExternalInput")
    atab_d = nc.dram_tensor("atab", [128, T * _BLK], mybir.dt.bfloat16,
                            kind="ExternalInput")
    invc_d = nc.dram_tensor("invc", [64, _RC], mybir.dt.bfloat16,
                            kind="ExternalInput")
    out = nc.dram_tensor("out", [64, _RC], mybir.dt.float32,
                         kind="ExternalOutput")

    with tile.TileContext(nc) as tc:
        with (
            tc.tile_pool(name="meta", bufs=1) as mpool,
            tc.tile_pool(name="stg", bufs=2, space="DRAM") as spool,
            tc.tile_pool(name="ga", bufs=4) as gapool,
            tc.tile_pool(name="gb", bufs=3) as gbpool,
            tc.tile_pool(name="gc", bufs=3) as gcpool,
            tc.tile_pool(name="ps", bufs=2, space="PSUM") as ppool,
        ):
            gidx_sb = mpool.tile([128, glen // 16], mybir.dt.int16, tag="gi")
            ridx_sb = mpool.tile([128, rlen // 16], mybir.dt.int16, tag="ri")
            atab_sb = mpool.tile([128, T, _BLK], mybir.dt.bfloat16, tag="at")
            invc_sb = mpool.tile([64, _RC], mybir.dt.bfloat16, tag="iv")
            outT = mpool.tile([64, _RC], mybir.dt.bfloat16, tag="oT")
            nc.sync.dma_start(out=gidx_sb[:], in_=gidx_d[:])
            nc.sync.dma_start(out=ridx_sb[:], in_=ridx_d[:])
            nc.sync.dma_start(
                out=atab_sb[:],
                in_=atab_d[:].rearrange("p (t m) -> p t m", m=_BLK))
            nc.sync.dma_start(out=invc_sb[:], in_=invc_d[:])

            for _i in range(4):
                gz = gapool.tile([128, (_CH // 128) * _D], mybir.dt.float32,
                                 tag="ga")
                nc.vector.memset(gz[:], 0.0)
            for _i in range(3):
                gz2 = gbpool.tile([128, (_CH // 128) * _D], mybir.dt.float32,
                                  tag="gb")
                nc.vector.memset(gz2[:], 0.0)

            for _rep in range(n_reps):
                stage = spool.tile([spad, _D], mybir.dt.float32, tag="st")
                # ---- phase A: window gather -> staging ----
                goff = 0
                wr = 0
                for (s, w, pos, n) in chunksA:
                    base = w * _WIN
                    wsize = min(_WIN, _V - base)
                    cc = n // 128
                    ga = gapool.tile([128, (_CH // 128) * _D],
                                     mybir.dt.float32, tag="ga")
                    nc.gpsimd.dma_gather(
                        out_ap=ga[:, :cc * _D].rearrange(
                            "p (cc d) -> p cc d", d=_D),
                        in_ap=table[base:base + wsize, :],
                        idxs_ap=gidx_sb[:, goff // 16:(goff + n) // 16],
                        num_idxs=n, num_idxs_reg=n,
                        elem_size=_D, queue_num=0,
                    )
                    # transposed-within-chunk staging: partition p holds
                    # staging rows [pos + p*cc, pos + (p+1)*cc) contiguously
                    dst = stage[pos:pos + n, :].rearrange(
                        "(p cc) d -> p (cc d)", p=128)
                    eng = nc.sync if wr % 8 < 5 else nc.scalar
                    eng.dma_start(out=dst, in_=ga[:, :cc * _D])
                    wr += 1
                    goff += n

                # ---- phase B: row-major gather + matmul combine ----
                roff = 0
                bi = 0          # index into chunksB
                gcur = None
                gbf = None
                gend = 0
                gbase = 0
                col = 0
                for g in range(_NGRP):
                    psum = ppool.tile([64, _BANK], mybir.dt.float32, tag="ps")
                    for bb in range(_BPG):
                        b = g * _BPG + bb
                        ntile = int(tiles_pb[b])
                        for t in range(ntile):
                            if col * 128 >= gend:
                                (s, p, n) = chunksB[bi]
                                bi += 1
                                gbase = p
                                gend = p + n
                                sb0 = sb_base[s]
                                sbn = min(sb_base[s + 1], spad) - sb0
                                gcur = gbpool.tile([128, (_CH // 128) * _D],
                                                   mybir.dt.float32, tag="gb")
                                nc.gpsimd.dma_gather(
                                    out_ap=gcur[:, :_cdiv(n, 128) * _D]
                                    .rearrange("p (cc d) -> p cc d", d=_D),
                                    in_ap=stage[sb0:sb0 + sbn, :],
                                    idxs_ap=ridx_sb[:, roff // 16:
                                                    (roff + n) // 16],
                                    num_idxs=n, num_idxs_reg=n,
                                    elem_size=_D, queue_num=0,
                                )
                                roff += n
                                gbf = gcpool.tile([128, (_CH // 128) * _D],
                                                  mybir.dt.bfloat16, tag="gc")
                                if bi % 2 == 0:
                                    nc.vector.tensor_copy(
                                        out=gbf[:], in_=gcur[:])
                                else:
                                    nc.scalar.copy(
                                        out=gbf[:], in_=gcur[:])
                            lc = col - gbase // 128
                            nc.tensor.matmul(
                                out=psum[:, bb * _BLK:(bb + 1) * _BLK],
                                lhsT=gbf[:, lc * _D:(lc + 1) * _D],
                                rhs=atab_sb[:, col, :],
                                start=(t == 0),
                                stop=(t == ntile - 1),
                            )
                            col += 1
                    nc.vector.tensor_tensor(
                        out=outT[:, g * _BANK:(g + 1) * _BANK],
                        in0=psum[:],
                        in1=invc_sb[:, g * _BANK:(g + 1) * _BANK],
                        op=mybir.AluOpType.mult,
                    )
            nc.gpsimd.dma_start(out=out[:], in_=outT[:])
    nc.compile()
    return nc


def _state(values, row_indices, emb_table, n_reps=1):
    struct, in_maps = _prep(values, row_indices)
    key = (struct, n_reps)
    if key not in _prog_cache:
        _prog_cache[key] = _build(struct, n_reps=n_reps)
    nc = _prog_cache[key]
    table = np.ascontiguousarray(np.asarray(emb_table, dtype=np.float32))
    for m in in_maps:
        m["table"] = table
    return nc, in_maps


def kernel(values, row_indices, emb_table):
    from concourse.bass_utils import run_bass_kernel_spmd

    nc, in_maps = _state(values, row_indices, emb_table)
    res = run_bass_kernel_spmd(nc, in_maps, core_ids=list(range(_M)))
    full = np.concatenate(
        [np.asarray(res.results[c]["out"]).T for c in range(_M)], axis=0)
    return np.ascontiguousarray(full.reshape(_B, _S, _D).astype(np.float32))


# revision 16
# speedup vs baseline: 5.5217x; 1.0351x over previous
"""Distributed sparse embedding lookup (mean combiner) on 8 Trainium2 cores.

Architecture (data-parallel over output rows, table replicated per core,
two-phase gather through an HBM staging buffer, matmul combine):

  - Each core owns 1/8 of the output rows (13312 = 208 blocks x 64 rows).
    row_indices is sorted, so each core's keys are a contiguous input slice.
  - Output slots: block b gets tiles_pb[b] = max-over-cores ceil(keys_b/128)
    tiles of 128 gather slots (row-major within the block, pad slots at the
    tile tails). Slots are grouped into SUPERBLOCKS of 28672 so that each
    superblock's staging region fits one int16 dma_gather window.
  - Phase A: per (superblock, table-window of 32768 vocab rows), dma_gather
    the slots' unique keys (f32 256B rows, <=1024 idxs per instruction — the
    HW-validated ring limit) and stream the result to an HBM staging buffer
    with dense HWDGE writes alternating between the SP and ACT engines.
    Staging order = (superblock, window, key) — affine from the gather.
  - Phase B: per slot chunk, dma_gather row-major from this superblock's
    staging region (idx = position of the slot's key inside the region),
    cast f32->bf16 on DVE, then one PE matmul per 128-slot tile:
    lhsT = gathered [128 keys, 64 emb] bf16, rhs = host-built one-hot
    A^T [128 keys, 64 rows-in-block] bf16, accumulated into a PSUM bank of
    512 rows (first tile of each block starts the window, so no PSUM
    zeroing). 1/count is applied per row during the PSUM->SBUF copy on DVE.
  - Staging is a DRAM tile pool with bufs=2 so reps pipeline.
  - Final: out.T SBUF bf16 -> DRAM f32 cast-DMA once; host transposes and
    concatenates the 8 core slabs.

All tile/budget bookkeeping is shared across cores (max over cores), so one
SPMD program serves all 8; per-core variation lives in uploaded tensors.
"""
import numpy as np
import ml_dtypes

_B, _S, _D = 4096, 26, 64
_V = 1_000_000
_M = 8
_R = _B * _S              # 106496 output rows
_RC = _R // _M            # 13312 rows per core
_BLK = 64                 # rows per block = one-hot width
_BANK = 512               # rows per PSUM bank group
_NBLK = _RC // _BLK       # 208
_NGRP = _RC // _BANK      # 26
_BPG = _BANK // _BLK      # 8 blocks per group
_WIN = 32768              # table window (int16 idx)
_NWIN = (_V + _WIN - 1) // _WIN
_SBS = 28672              # slots per superblock (28 * 1024)
_CH = 1024                # keys per dma_gather instruction
_BF16 = ml_dtypes.bfloat16

_prog_cache = {}


def _cdiv(a, b):
    return (a + b - 1) // b


def _pack16(v, budget, pad):
    out = np.full(budget, pad, dtype=np.int16)
    out[: len(v)] = v
    return np.tile(out.reshape(-1, 16).T, (8, 1))


def _prep(values, row_indices):
    values = np.asarray(values).astype(np.int64)
    row_indices = np.asarray(row_indices).astype(np.int64)
    if np.any(np.diff(row_indices) < 0):
        order = np.argsort(row_indices, kind="stable")
        values, row_indices = values[order], row_indices[order]
    bounds = np.searchsorted(row_indices, np.arange(_M + 1) * _RC)

    cores = []
    kcb = np.zeros((_M, _NBLK), dtype=np.int64)
    for c in range(_M):
        lo, hi = bounds[c], bounds[c + 1]
        keys = values[lo:hi]
        rows = row_indices[lo:hi] - c * _RC
        kcb[c] = np.bincount(rows // _BLK, minlength=_NBLK)
        cores.append((keys, rows))

    tiles_pb = np.maximum(1, -(-kcb.max(axis=0) // 128))
    col0 = np.concatenate([[0], np.cumsum(tiles_pb)])
    T = int(col0[-1])
    nslot = T * 128
    nsb = _cdiv(nslot, _SBS)

    # per core: slot -> (key or -1); slot superblock id
    slot_key = np.full((_M, nslot), -1, dtype=np.int64)
    slot_row = np.full((_M, nslot), 0, dtype=np.int64)
    for c in range(_M):
        keys, rows = cores[c]
        n = len(keys)
        blk = rows // _BLK
        bstart = np.searchsorted(blk, np.arange(_NBLK))
        islot = (col0[blk] * 128 + (np.arange(n) - bstart[blk]))
        slot_key[c, islot] = keys
        slot_row[c, islot] = rows

    slot_sb = np.arange(nslot) // _SBS
    slot_win = np.where(slot_key >= 0, slot_key // _WIN, -1)   # [M, nslot]

    # phase-A budgets per (superblock, window): unique keys, max over cores,
    # rounded to 128
    uniq = {}      # (c, s, w) -> (unique_keys_ascending, slot_positions_map)
    bud = np.zeros((nsb, _NWIN), dtype=np.int64)
    for c in range(_M):
        for s in range(nsb):
            m_s = slot_sb == s
            for w in range(_NWIN):
                m = m_s & (slot_win[c] == w)
                u = np.unique(slot_key[c][m])
                uniq[(c, s, w)] = u
                bud[s, w] = max(bud[s, w], len(u))
    bud = -(-bud // 128) * 128
    sb_size = bud.sum(axis=1)              # staging rows per superblock
    assert sb_size.max() <= _WIN, sb_size
    sb_base = np.concatenate([[0], np.cumsum(sb_size)])
    spad = int(sb_base[-1])

    # phase-A chunk list (shared): (s, w, staging_pos, n_budget) split <=CH
    chunksA = []
    for s in range(nsb):
        pos = int(sb_base[s])
        for w in range(_NWIN):
            left = int(bud[s, w])
            while left > 0:
                n = min(_CH, left)
                chunksA.append((s, w, pos, n))
                pos += n
                left -= n
    # phase-B chunk list: (s, slot_start, n_slots) split <=CH within sblocks
    chunksB = []
    for s in range(nsb):
        st = s * _SBS
        en = min(nslot, (s + 1) * _SBS)
        p = st
        while p < en:
            n = min(_CH, en - p)
            chunksB.append((s, p, n))
            p += n

    in_maps = []
    for c in range(_M):
        # staging position of each (s, w) unique key. Within each <=1024-key
        # gather chunk the staging order is TRANSPOSED (partition-major:
        # local key j -> (j%128)*(n/128) + j//128) so the HWDGE staging
        # write is one contiguous run per partition.
        posmap = {}
        gidx_parts = []
        wpos = {}
        for s in range(nsb):
            pos = int(sb_base[s])
            for w in range(_NWIN):
                u = uniq[(c, s, w)]
                wpos[(s, w)] = pos
                b_sw = int(bud[s, w])
                for i, k in enumerate(u):
                    ck = i // _CH
                    j = i % _CH
                    n_k = min(_CH, b_sw - ck * _CH)
                    posmap[(s, k)] = (pos + ck * _CH
                                      + (j % 128) * (n_k // 128) + j // 128)
                pos += b_sw
        # gidx stream per phase-A chunk (local window idx, pad=0)
        for (s, w, pos, n) in chunksA:
            u = uniq[(c, s, w)]
            off = pos - wpos[(s, w)]
            seg = (u[off:off + n] - w * _WIN).astype(np.int16)
            gidx_parts.append(_pack16(seg, n, np.int16(0)))
        gidx = np.concatenate(gidx_parts, axis=1)

        # ridx stream per phase-B chunk (position within superblock region)
        ridx_parts = []
        for (s, p, n) in chunksB:
            sk = slot_key[c, p:p + n]
            loc = np.zeros(n, dtype=np.int16)
            nzm = sk >= 0
            if nzm.any():
                loc[nzm] = np.array(
                    [posmap[(s, k)] for k in sk[nzm]],
                    dtype=np.int64) - sb_base[s]
            ridx_parts.append(_pack16(loc, n, np.int16(0)))
        ridx = np.concatenate(ridx_parts, axis=1)

        # A^T one-hot [128, T*BLK] bf16  (slot -> row-in-block)
        atab = np.zeros((128, T * _BLK), dtype=np.float32)
        sl = np.arange(nslot)
        colv = sl // 128
        pv = sl % 128
        realm = slot_key[c] >= 0
        blkv = np.zeros(nslot, dtype=np.int64)
        blkv[realm] = slot_row[c][realm] // _BLK
        atab[pv[realm], colv[realm] * _BLK
             + (slot_row[c][realm] - blkv[realm] * _BLK)] = 1.0

        keys, rows = cores[c]
        counts = np.bincount(rows, minlength=_RC).astype(np.float32)
        invc = 1.0 / np.maximum(counts, 1.0)

        in_maps.append({
            "gidx": np.ascontiguousarray(gidx),
            "ridx": np.ascontiguousarray(ridx),
            "atab": atab.astype(_BF16),
            "invc": np.broadcast_to(invc.astype(_BF16), (64, _RC)).copy(),
        })
    struct = (tuple(int(x) for x in tiles_pb), tuple(chunksA),
              tuple(chunksB), spad, tuple(int(x) for x in sb_base))
    return struct, in_maps


def _build(struct, n_reps=1):
    from concourse import bacc, mybir, tile

    tiles_pb, chunksA, chunksB, spad, sb_base = struct
    T = sum(tiles_pb)
    glen = sum(n for (_s, _w, _p, n) in chunksA)
    rlen = sum(n for (_s, _p, n) in chunksB)

    nc = bacc.Bacc(None, target_bir_lowering=False, debug=False,
                   num_swdge_queues=1)
    table = nc.dram_tensor("table", [_V, _D], mybir.dt.float32,
                           kind="ExternalInput")
    gidx_d = nc.dram_tensor("gidx", [128, glen // 16], mybir.dt.int16,
                            kind="ExternalInput")
    ridx_d = nc.dram_tensor("ridx", [128, rlen // 16], mybir.dt.int16,
                            kind="ExternalInput")
    atab_d = nc.dram_tensor("atab", [128, T * _BLK], mybir.dt.bfloat16,
                            kind="ExternalInput")
    invc_d = nc.dram_tensor("invc", [64, _RC], mybir.dt.bfloat16,
                            kind="ExternalInput")
    out = nc.dram_tensor("out", [64, _RC], mybir.dt.float32,
                         kind="ExternalOutput")

    with tile.TileContext(nc) as tc:
        with (
            tc.tile_pool(name="meta", bufs=1) as mpool,
            tc.tile_pool(name="stg", bufs=3, space="DRAM") as spool,
            tc.tile_pool(name="ga", bufs=6) as gapool,
            tc.tile_pool(name="gb", bufs=4) as gbpool,
            tc.tile_pool(name="gc", bufs=3) as gcpool,
            tc.tile_pool(name="ps", bufs=2, space="PSUM") as ppool,
        ):
            gidx_sb = mpool.tile([128, glen // 16], mybir.dt.int16, tag="gi")
            ridx_sb = mpool.tile([128, rlen // 16], mybir.dt.int16, tag="ri")
            atab_sb = mpool.tile([128, T, _BLK], mybir.dt.bfloat16, tag="at")
            invc_sb = mpool.tile([64, _RC], mybir.dt.bfloat16, tag="iv")
            outT = mpool.tile([64, _RC], mybir.dt.bfloat16, tag="oT")
            nc.sync.dma_start(out=gidx_sb[:], in_=gidx_d[:])
            nc.sync.dma_start(out=ridx_sb[:], in_=ridx_d[:])
            nc.sync.dma_start(
                out=atab_sb[:],
                in_=atab_d[:].rearrange("p (t m) -> p t m", m=_BLK))
            nc.sync.dma_start(out=invc_sb[:], in_=invc_d[:])

            for _i in range(6):
                gz = gapool.tile([128, (_CH // 128) * _D], mybir.dt.float32,
                                 tag="ga")
                nc.vector.memset(gz[:], 0.0)
            for _i in range(4):
                gz2 = gbpool.tile([128, (_CH // 128) * _D], mybir.dt.float32,
                                  tag="gb")
                nc.vector.memset(gz2[:], 0.0)

            for _rep in range(n_reps):
                stage = spool.tile([spad, _D], mybir.dt.float32, tag="st")
                # ---- phase A: window gather -> staging ----
                goff = 0
                wr = 0
                for (s, w, pos, n) in chunksA:
                    base = w * _WIN
                    wsize = min(_WIN, _V - base)
                    cc = n // 128
                    ga = gapool.tile([128, (_CH // 128) * _D],
                                     mybir.dt.float32, tag="ga")
                    nc.gpsimd.dma_gather(
                        out_ap=ga[:, :cc * _D].rearrange(
                            "p (cc d) -> p cc d", d=_D),
                        in_ap=table[base:base + wsize, :],
                        idxs_ap=gidx_sb[:, goff // 16:(goff + n) // 16],
                        num_idxs=n, num_idxs_reg=n,
                        elem_size=_D, queue_num=0,
                    )
                    # transposed-within-chunk staging: partition p holds
                    # staging rows [pos + p*cc, pos + (p+1)*cc) contiguously
                    dst = stage[pos:pos + n, :].rearrange(
                        "(p cc) d -> p (cc d)", p=128)
                    eng = nc.sync if wr % 8 < 5 else nc.scalar
                    eng.dma_start(out=dst, in_=ga[:, :cc * _D])
                    wr += 1
                    goff += n

                # ---- phase B: row-major gather + matmul combine ----
                roff = 0
                bi = 0          # index into chunksB
                gcur = None
                gbf = None
                gend = 0
                gbase = 0
                col = 0
                for g in range(_NGRP):
                    psum = ppool.tile([64, _BANK], mybir.dt.float32, tag="ps")
                    for bb in range(_BPG):
                        b = g * _BPG + bb
                        ntile = int(tiles_pb[b])
                        for t in range(ntile):
                            if col * 128 >= gend:
                                (s, p, n) = chunksB[bi]
                                bi += 1
                                gbase = p
                                gend = p + n
                                sb0 = sb_base[s]
                                sbn = min(sb_base[s + 1], spad) - sb0
                                gcur = gbpool.tile([128, (_CH // 128) * _D],
                                                   mybir.dt.float32, tag="gb")
                                nc.gpsimd.dma_gather(
                                    out_ap=gcur[:, :_cdiv(n, 128) * _D]
                                    .rearrange("p (cc d) -> p cc d", d=_D),
                                    in_ap=stage[sb0:sb0 + sbn, :],
                                    idxs_ap=ridx_sb[:, roff // 16:
                                                    (roff + n) // 16],
                                    num_idxs=n, num_idxs_reg=n,
                                    elem_size=_D, queue_num=0,
                                )
                                roff += n
                                gbf = gcpool.tile([128, (_CH // 128) * _D],
                                                  mybir.dt.bfloat16, tag="gc")
                                if bi % 2 == 0:
                                    nc.vector.tensor_copy(
                                        out=gbf[:], in_=gcur[:])
                                else:
                                    nc.scalar.copy(
                                        out=gbf[:], in_=gcur[:])
                            lc = col - gbase // 128
                            nc.tensor.matmul(
                                out=psum[:, bb * _BLK:(bb + 1) * _BLK],
                                lhsT=gbf[:, lc * _D:(lc + 1) * _D],
                                rhs=atab_sb[:, col, :],
                                start=(t == 0),
                                stop=(t == ntile - 1),
                            )
                            col += 1
                    nc.vector.tensor_tensor(
                        out=outT[:, g * _BANK:(g + 1) * _BANK],
                        in0=psum[:],
                        in1=invc_sb[:, g * _BANK:(g + 1) * _BANK],
                        op=mybir.AluOpType.mult,
                    )
            nc.gpsimd.dma_start(out=out[:], in_=outT[:])
    nc.compile()
    return nc


def _state(values, row_indices, emb_table, n_reps=1):
    struct, in_maps = _prep(values, row_indices)
    key = (struct, n_reps)
    if key not in _prog_cache:
        _prog_cache[key] = _build(struct, n_reps=n_reps)
    nc = _prog_cache[key]
    table = np.ascontiguousarray(np.asarray(emb_table, dtype=np.float32))
    for m in in_maps:
        m["table"] = table
    return nc, in_maps


def kernel(values, row_indices, emb_table):
    from concourse.bass_utils import run_bass_kernel_spmd

    nc, in_maps = _state(values, row_indices, emb_table)
    res = run_bass_kernel_spmd(nc, in_maps, core_ids=list(range(_M)))
    full = np.concatenate(
        [np.asarray(res.results[c]["out"]).T for c in range(_M)], axis=0)
    return np.ascontiguousarray(full.reshape(_B, _S, _D).astype(np.float32))


# revision 18
# speedup vs baseline: 5.6853x; 1.0296x over previous
"""Distributed sparse embedding lookup (mean combiner) on 8 Trainium2 cores.

Architecture (data-parallel over output rows, table replicated per core,
two-phase gather through an HBM staging buffer, matmul combine):

  - Each core owns 1/8 of the output rows (13312 = 208 blocks x 64 rows).
    row_indices is sorted, so each core's keys are a contiguous input slice.
  - Output slots: block b gets tiles_pb[b] = max-over-cores ceil(keys_b/128)
    tiles of 128 gather slots (row-major within the block, pad slots at the
    tile tails). Slots are grouped into SUPERBLOCKS of 28672 so that each
    superblock's staging region fits one int16 dma_gather window.
  - Phase A: per (superblock, table-window of 32768 vocab rows), dma_gather
    the slots' unique keys (f32 256B rows, <=1024 idxs per instruction — the
    HW-validated ring limit) and stream the result to an HBM staging buffer
    with dense HWDGE writes alternating between the SP and ACT engines.
    Staging order = (superblock, window, key) — affine from the gather.
  - Phase B: per slot chunk, dma_gather row-major from this superblock's
    staging region (idx = position of the slot's key inside the region),
    cast f32->bf16 on DVE, then one PE matmul per 128-slot tile:
    lhsT = gathered [128 keys, 64 emb] bf16, rhs = host-built one-hot
    A^T [128 keys, 64 rows-in-block] bf16, accumulated into a PSUM bank of
    512 rows (first tile of each block starts the window, so no PSUM
    zeroing). 1/count is applied per row during the PSUM->SBUF copy on DVE.
  - Staging is a DRAM tile pool with bufs=2 so reps pipeline.
  - Final: out.T SBUF bf16 -> DRAM f32 cast-DMA once; host transposes and
    concatenates the 8 core slabs.

All tile/budget bookkeeping is shared across cores (max over cores), so one
SPMD program serves all 8; per-core variation lives in uploaded tensors.
"""
import numpy as np
import ml_dtypes

_B, _S, _D = 4096, 26, 64
_V = 1_000_000
_M = 8
_R = _B * _S              # 106496 output rows
_RC = _R // _M            # 13312 rows per core
_BLK = 64                 # rows per block = one-hot width
_BANK = 512               # rows per PSUM bank group
_NBLK = _RC // _BLK       # 208
_NGRP = _RC // _BANK      # 26
_BPG = _BANK // _BLK      # 8 blocks per group
_WIN = 32768              # table window (int16 idx)
_NWIN = (_V + _WIN - 1) // _WIN
_SBS = 28672              # slots per superblock (28 * 1024)
_CH = 1024                # keys per phase-A dma_gather instruction
_CHB = 512                # slots per phase-B dma_gather instruction
_BF16 = ml_dtypes.bfloat16

_prog_cache = {}


def _cdiv(a, b):
    return (a + b - 1) // b


def _pack16(v, budget, pad):
    out = np.full(budget, pad, dtype=np.int16)
    out[: len(v)] = v
    return np.tile(out.reshape(-1, 16).T, (8, 1))


def _prep(values, row_indices):
    values = np.asarray(values).astype(np.int64)
    row_indices = np.asarray(row_indices).astype(np.int64)
    if np.any(np.diff(row_indices) < 0):
        order = np.argsort(row_indices, kind="stable")
        values, row_indices = values[order], row_indices[order]
    bounds = np.searchsorted(row_indices, np.arange(_M + 1) * _RC)

    cores = []
    kcb = np.zeros((_M, _NBLK), dtype=np.int64)
    for c in range(_M):
        lo, hi = bounds[c], bounds[c + 1]
        keys = values[lo:hi]
        rows = row_indices[lo:hi] - c * _RC
        kcb[c] = np.bincount(rows // _BLK, minlength=_NBLK)
        cores.append((keys, rows))

    tiles_pb = np.maximum(1, -(-kcb.max(axis=0) // 128))
    col0 = np.concatenate([[0], np.cumsum(tiles_pb)])
    T = int(col0[-1])
    nslot = T * 128
    nsb = _cdiv(nslot, _SBS)

    # per core: slot -> (key or -1); slot superblock id
    slot_key = np.full((_M, nslot), -1, dtype=np.int64)
    slot_row = np.full((_M, nslot), 0, dtype=np.int64)
    for c in range(_M):
        keys, rows = cores[c]
        n = len(keys)
        blk = rows // _BLK
        bstart = np.searchsorted(blk, np.arange(_NBLK))
        islot = (col0[blk] * 128 + (np.arange(n) - bstart[blk]))
        slot_key[c, islot] = keys
        slot_row[c, islot] = rows

    slot_sb = np.arange(nslot) // _SBS
    slot_win = np.where(slot_key >= 0, slot_key // _WIN, -1)   # [M, nslot]

    # phase-A budgets per (superblock, window): unique keys, max over cores,
    # rounded to 128
    uniq = {}      # (c, s, w) -> (unique_keys_ascending, slot_positions_map)
    bud = np.zeros((nsb, _NWIN), dtype=np.int64)
    for c in range(_M):
        for s in range(nsb):
            m_s = slot_sb == s
            for w in range(_NWIN):
                m = m_s & (slot_win[c] == w)
                u = np.unique(slot_key[c][m])
                uniq[(c, s, w)] = u
                bud[s, w] = max(bud[s, w], len(u))
    bud = -(-bud // 128) * 128
    sb_size = bud.sum(axis=1)              # staging rows per superblock
    assert sb_size.max() <= _WIN, sb_size
    sb_base = np.concatenate([[0], np.cumsum(sb_size)])
    spad = int(sb_base[-1])

    # phase-A chunk list (shared): (s, w, staging_pos, n_budget) split <=CH
    chunksA = []
    for s in range(nsb):
        pos = int(sb_base[s])
        for w in range(_NWIN):
            left = int(bud[s, w])
            while left > 0:
                n = min(_CH, left)
                chunksA.append((s, w, pos, n))
                pos += n
                left -= n
    # phase-B chunk list: (s, slot_start, n_slots) split <=CHB within sblocks
    chunksB = []
    for s in range(nsb):
        st = s * _SBS
        en = min(nslot, (s + 1) * _SBS)
        p = st
        while p < en:
            n = min(_CHB, en - p)
            chunksB.append((s, p, n))
            p += n

    in_maps = []
    for c in range(_M):
        # staging position of each (s, w) unique key. Within each <=1024-key
        # gather chunk the staging order is TRANSPOSED (partition-major:
        # local key j -> (j%128)*(n/128) + j//128) so the HWDGE staging
        # write is one contiguous run per partition.
        posmap = {}
        gidx_parts = []
        wpos = {}
        for s in range(nsb):
            pos = int(sb_base[s])
            for w in range(_NWIN):
                u = uniq[(c, s, w)]
                wpos[(s, w)] = pos
                b_sw = int(bud[s, w])
                for i, k in enumerate(u):
                    ck = i // _CH
                    j = i % _CH
                    n_k = min(_CH, b_sw - ck * _CH)
                    posmap[(s, k)] = (pos + ck * _CH
                                      + (j % 128) * (n_k // 128) + j // 128)
                pos += b_sw
        # gidx stream per phase-A chunk (local window idx, pad=0)
        for (s, w, pos, n) in chunksA:
            u = uniq[(c, s, w)]
            off = pos - wpos[(s, w)]
            seg = (u[off:off + n] - w * _WIN).astype(np.int16)
            gidx_parts.append(_pack16(seg, n, np.int16(0)))
        gidx = np.concatenate(gidx_parts, axis=1)

        # ridx stream per phase-B chunk (position within superblock region)
        ridx_parts = []
        for (s, p, n) in chunksB:
            sk = slot_key[c, p:p + n]
            loc = np.zeros(n, dtype=np.int16)
            nzm = sk >= 0
            if nzm.any():
                loc[nzm] = np.array(
                    [posmap[(s, k)] for k in sk[nzm]],
                    dtype=np.int64) - sb_base[s]
            ridx_parts.append(_pack16(loc, n, np.int16(0)))
        ridx = np.concatenate(ridx_parts, axis=1)

        # A^T one-hot [128, T*BLK] bf16  (slot -> row-in-block)
        atab = np.zeros((128, T * _BLK), dtype=np.float32)
        sl = np.arange(nslot)
        colv = sl // 128
        pv = sl % 128
        realm = slot_key[c] >= 0
        blkv = np.zeros(nslot, dtype=np.int64)
        blkv[realm] = slot_row[c][realm] // _BLK
        atab[pv[realm], colv[realm] * _BLK
             + (slot_row[c][realm] - blkv[realm] * _BLK)] = 1.0

        keys, rows = cores[c]
        counts = np.bincount(rows, minlength=_RC).astype(np.float32)
        invc = 1.0 / np.maximum(counts, 1.0)

        in_maps.append({
            "gidx": np.ascontiguousarray(gidx),
            "ridx": np.ascontiguousarray(ridx),
            "atab": atab.astype(_BF16),
            "invc": np.broadcast_to(invc.astype(_BF16), (64, _RC)).copy(),
        })
    struct = (tuple(int(x) for x in tiles_pb), tuple(chunksA),
              tuple(chunksB), spad, tuple(int(x) for x in sb_base))
    return struct, in_maps


def _build(struct, n_reps=1):
    from concourse import bacc, mybir, tile

    tiles_pb, chunksA, chunksB, spad, sb_base = struct
    T = sum(tiles_pb)
    glen = sum(n for (_s, _w, _p, n) in chunksA)
    rlen = sum(n for (_s, _p, n) in chunksB)

    nc = bacc.Bacc(None, target_bir_lowering=False, debug=False,
                   num_swdge_queues=1)
    table = nc.dram_tensor("table", [_V, _D], mybir.dt.float32,
                           kind="ExternalInput")
    gidx_d = nc.dram_tensor("gidx", [128, glen // 16], mybir.dt.int16,
                            kind="ExternalInput")
    ridx_d = nc.dram_tensor("ridx", [128, rlen // 16], mybir.dt.int16,
                            kind="ExternalInput")
    atab_d = nc.dram_tensor("atab", [128, T * _BLK], mybir.dt.bfloat16,
                            kind="ExternalInput")
    invc_d = nc.dram_tensor("invc", [64, _RC], mybir.dt.bfloat16,
                            kind="ExternalInput")
    out = nc.dram_tensor("out", [64, _RC], mybir.dt.float32,
                         kind="ExternalOutput")

    with tile.TileContext(nc) as tc:
        with (
            tc.tile_pool(name="meta", bufs=1) as mpool,
            tc.tile_pool(name="stg", bufs=3, space="DRAM") as spool,
            tc.tile_pool(name="ga", bufs=6) as gapool,
            tc.tile_pool(name="gb", bufs=4) as gbpool,
            tc.tile_pool(name="gc", bufs=3) as gcpool,
            tc.tile_pool(name="ps", bufs=2, space="PSUM") as ppool,
        ):
            gidx_sb = mpool.tile([128, glen // 16], mybir.dt.int16, tag="gi")
            ridx_sb = mpool.tile([128, rlen // 16], mybir.dt.int16, tag="ri")
            atab_sb = mpool.tile([128, T, _BLK], mybir.dt.bfloat16, tag="at")
            invc_sb = mpool.tile([64, _RC], mybir.dt.bfloat16, tag="iv")
            outT = mpool.tile([64, _RC], mybir.dt.bfloat16, tag="oT")
            nc.sync.dma_start(out=gidx_sb[:], in_=gidx_d[:])
            nc.sync.dma_start(out=ridx_sb[:], in_=ridx_d[:])
            nc.sync.dma_start(
                out=atab_sb[:],
                in_=atab_d[:].rearrange("p (t m) -> p t m", m=_BLK))
            nc.sync.dma_start(out=invc_sb[:], in_=invc_d[:])

            for _i in range(6):
                gz = gapool.tile([128, (_CH // 128) * _D], mybir.dt.float32,
                                 tag="ga")
                nc.vector.memset(gz[:], 0.0)
            for _i in range(4):
                gz2 = gbpool.tile([128, (_CHB // 128) * _D], mybir.dt.float32,
                                  tag="gb")
                nc.vector.memset(gz2[:], 0.0)

            for _rep in range(n_reps):
                stage = spool.tile([spad, _D], mybir.dt.float32, tag="st")
                # ---- phase A: window gather -> staging ----
                goff = 0
                wr = 0
                for (s, w, pos, n) in chunksA:
                    base = w * _WIN
                    wsize = min(_WIN, _V - base)
                    cc = n // 128
                    ga = gapool.tile([128, (_CH // 128) * _D],
                                     mybir.dt.float32, tag="ga")
                    nc.gpsimd.dma_gather(
                        out_ap=ga[:, :cc * _D].rearrange(
                            "p (cc d) -> p cc d", d=_D),
                        in_ap=table[base:base + wsize, :],
                        idxs_ap=gidx_sb[:, goff // 16:(goff + n) // 16],
                        num_idxs=n, num_idxs_reg=n,
                        elem_size=_D, queue_num=0,
                    )
                    # transposed-within-chunk staging: partition p holds
                    # staging rows [pos + p*cc, pos + (p+1)*cc) contiguously
                    dst = stage[pos:pos + n, :].rearrange(
                        "(p cc) d -> p (cc d)", p=128)
                    eng = nc.sync if wr % 8 < 5 else nc.scalar
                    eng.dma_start(out=dst, in_=ga[:, :cc * _D])
                    wr += 1
                    goff += n

                # ---- phase B: row-major gather + matmul combine ----
                roff = 0
                bi = 0          # index into chunksB
                gcur = None
                gbf = None
                gend = 0
                gbase = 0
                col = 0
                for g in range(_NGRP):
                    psum = ppool.tile([64, _BANK], mybir.dt.float32, tag="ps")
                    for bb in range(_BPG):
                        b = g * _BPG + bb
                        ntile = int(tiles_pb[b])
                        for t in range(ntile):
                            if col * 128 >= gend:
                                (s, p, n) = chunksB[bi]
                                bi += 1
                                gbase = p
                                gend = p + n
                                sb0 = sb_base[s]
                                sbn = min(sb_base[s + 1], spad) - sb0
                                gcur = gbpool.tile([128, (_CHB // 128) * _D],
                                                   mybir.dt.float32, tag="gb")
                                nc.gpsimd.dma_gather(
                                    out_ap=gcur[:, :_cdiv(n, 128) * _D]
                                    .rearrange("p (cc d) -> p cc d", d=_D),
                                    in_ap=stage[sb0:sb0 + sbn, :],
                                    idxs_ap=ridx_sb[:, roff // 16:
                                                    (roff + n) // 16],
                                    num_idxs=n, num_idxs_reg=n,
                                    elem_size=_D, queue_num=0,
                                )
                                roff += n
                                gbf = gcpool.tile([128, (_CHB // 128) * _D],
                                                  mybir.dt.bfloat16, tag="gc")
                                if bi % 2 == 0:
                                    nc.vector.tensor_copy(
                                        out=gbf[:], in_=gcur[:])
                                else:
                                    nc.scalar.copy(
                                        out=gbf[:], in_=gcur[:])
                            lc = col - gbase // 128
                            nc.tensor.matmul(
                                out=psum[:, bb * _BLK:(bb + 1) * _BLK],
                                lhsT=gbf[:, lc * _D:(lc + 1) * _D],
                                rhs=atab_sb[:, col, :],
                                start=(t == 0),
                                stop=(t == ntile - 1),
                            )
                            col += 1
                    nc.vector.tensor_tensor(
                        out=outT[:, g * _BANK:(g + 1) * _BANK],
                        in0=psum[:],
                        in1=invc_sb[:, g * _BANK:(g + 1) * _BANK],
                        op=mybir.AluOpType.mult,
                    )
            nc.gpsimd.dma_start(out=out[:], in_=outT[:])
    nc.compile()
    return nc


def _state(values, row_indices, emb_table, n_reps=1):
    struct, in_maps = _prep(values, row_indices)
    key = (struct, n_reps)
    if key not in _prog_cache:
        _prog_cache[key] = _build(struct, n_reps=n_reps)
    nc = _prog_cache[key]
    table = np.ascontiguousarray(np.asarray(emb_table, dtype=np.float32))
    for m in in_maps:
        m["table"] = table
    return nc, in_maps


def kernel(values, row_indices, emb_table):
    from concourse.bass_utils import run_bass_kernel_spmd

    nc, in_maps = _state(values, row_indices, emb_table)
    res = run_bass_kernel_spmd(nc, in_maps, core_ids=list(range(_M)))
    full = np.concatenate(
        [np.asarray(res.results[c]["out"]).T for c in range(_M)], axis=0)
    return np.ascontiguousarray(full.reshape(_B, _S, _D).astype(np.float32))


# revision 19
# speedup vs baseline: 5.7965x; 1.0196x over previous
"""Distributed sparse embedding lookup (mean combiner) on 8 Trainium2 cores.

Architecture (data-parallel over output rows, table replicated per core,
two-phase gather through an HBM staging buffer, matmul combine):

  - Each core owns 1/8 of the output rows (13312 = 208 blocks x 64 rows).
    row_indices is sorted, so each core's keys are a contiguous input slice.
  - Output slots: block b gets tiles_pb[b] = max-over-cores ceil(keys_b/128)
    tiles of 128 gather slots (row-major within the block, pad slots at the
    tile tails). Slots are grouped into SUPERBLOCKS of 28672 so that each
    superblock's staging region fits one int16 dma_gather window.
  - Phase A: per (superblock, table-window of 32768 vocab rows), dma_gather
    the slots' unique keys (f32 256B rows, <=1024 idxs per instruction — the
    HW-validated ring limit) and stream the result to an HBM staging buffer
    with dense HWDGE writes alternating between the SP and ACT engines.
    Staging order = (superblock, window, key) — affine from the gather.
  - Phase B: per slot chunk, dma_gather row-major from this superblock's
    staging region (idx = position of the slot's key inside the region),
    cast f32->bf16 on DVE, then one PE matmul per 128-slot tile:
    lhsT = gathered [128 keys, 64 emb] bf16, rhs = host-built one-hot
    A^T [128 keys, 64 rows-in-block] bf16, accumulated into a PSUM bank of
    512 rows (first tile of each block starts the window, so no PSUM
    zeroing). 1/count is applied per row during the PSUM->SBUF copy on DVE.
  - Staging is a DRAM tile pool with bufs=2 so reps pipeline.
  - Final: out.T SBUF bf16 -> DRAM f32 cast-DMA once; host transposes and
    concatenates the 8 core slabs.

All tile/budget bookkeeping is shared across cores (max over cores), so one
SPMD program serves all 8; per-core variation lives in uploaded tensors.
"""
import numpy as np
import ml_dtypes

_B, _S, _D = 4096, 26, 64
_V = 1_000_000
_M = 8
_R = _B * _S              # 106496 output rows
_RC = _R // _M            # 13312 rows per core
_BLK = 64                 # rows per block = one-hot width
_BANK = 512               # rows per PSUM bank group
_NBLK = _RC // _BLK       # 208
_NGRP = _RC // _BANK      # 26
_BPG = _BANK // _BLK      # 8 blocks per group
_WIN = 32768              # table window (int16 idx)
_NWIN = (_V + _WIN - 1) // _WIN
_SBS = 28672              # slots per superblock (28 * 1024)
_CH = 1024                # keys per phase-A dma_gather instruction
_CHB = 384                # slots per phase-B dma_gather instruction
_BF16 = ml_dtypes.bfloat16

_prog_cache = {}


def _cdiv(a, b):
    return (a + b - 1) // b


def _pack16(v, budget, pad):
    out = np.full(budget, pad, dtype=np.int16)
    out[: len(v)] = v
    return np.tile(out.reshape(-1, 16).T, (8, 1))


def _prep(values, row_indices):
    values = np.asarray(values).astype(np.int64)
    row_indices = np.asarray(row_indices).astype(np.int64)
    if np.any(np.diff(row_indices) < 0):
        order = np.argsort(row_indices, kind="stable")
        values, row_indices = values[order], row_indices[order]
    bounds = np.searchsorted(row_indices, np.arange(_M + 1) * _RC)

    cores = []
    kcb = np.zeros((_M, _NBLK), dtype=np.int64)
    for c in range(_M):
        lo, hi = bounds[c], bounds[c + 1]
        keys = values[lo:hi]
        rows = row_indices[lo:hi] - c * _RC
        kcb[c] = np.bincount(rows // _BLK, minlength=_NBLK)
        cores.append((keys, rows))

    tiles_pb = np.maximum(1, -(-kcb.max(axis=0) // 128))
    col0 = np.concatenate([[0], np.cumsum(tiles_pb)])
    T = int(col0[-1])
    nslot = T * 128
    nsb = _cdiv(nslot, _SBS)

    # per core: slot -> (key or -1); slot superblock id
    slot_key = np.full((_M, nslot), -1, dtype=np.int64)
    slot_row = np.full((_M, nslot), 0, dtype=np.int64)
    for c in range(_M):
        keys, rows = cores[c]
        n = len(keys)
        blk = rows // _BLK
        bstart = np.searchsorted(blk, np.arange(_NBLK))
        islot = (col0[blk] * 128 + (np.arange(n) - bstart[blk]))
        slot_key[c, islot] = keys
        slot_row[c, islot] = rows

    slot_sb = np.arange(nslot) // _SBS
    slot_win = np.where(slot_key >= 0, slot_key // _WIN, -1)   # [M, nslot]

    # phase-A budgets per (superblock, window): unique keys, max over cores,
    # rounded to 128
    uniq = {}      # (c, s, w) -> (unique_keys_ascending, slot_positions_map)
    bud = np.zeros((nsb, _NWIN), dtype=np.int64)
    for c in range(_M):
        for s in range(nsb):
            m_s = slot_sb == s
            for w in range(_NWIN):
                m = m_s & (slot_win[c] == w)
                u = np.unique(slot_key[c][m])
                uniq[(c, s, w)] = u
                bud[s, w] = max(bud[s, w], len(u))
    bud = -(-bud // 128) * 128
    sb_size = bud.sum(axis=1)              # staging rows per superblock
    assert sb_size.max() <= _WIN, sb_size
    sb_base = np.concatenate([[0], np.cumsum(sb_size)])
    spad = int(sb_base[-1])

    # phase-A chunk list (shared): (s, w, staging_pos, n_budget) split <=CH
    chunksA = []
    for s in range(nsb):
        pos = int(sb_base[s])
        for w in range(_NWIN):
            left = int(bud[s, w])
            while left > 0:
                n = min(_CH, left)
                chunksA.append((s, w, pos, n))
                pos += n
                left -= n
    # phase-B chunk list: (s, slot_start, n_slots) split <=CHB within sblocks
    chunksB = []
    for s in range(nsb):
        st = s * _SBS
        en = min(nslot, (s + 1) * _SBS)
        p = st
        while p < en:
            n = min(_CHB, en - p)
            chunksB.append((s, p, n))
            p += n

    in_maps = []
    for c in range(_M):
        # staging position of each (s, w) unique key. Within each <=1024-key
        # gather chunk the staging order is TRANSPOSED (partition-major:
        # local key j -> (j%128)*(n/128) + j//128) so the HWDGE staging
        # write is one contiguous run per partition.
        posmap = {}
        gidx_parts = []
        wpos = {}
        for s in range(nsb):
            pos = int(sb_base[s])
            for w in range(_NWIN):
                u = uniq[(c, s, w)]
                wpos[(s, w)] = pos
                b_sw = int(bud[s, w])
                for i, k in enumerate(u):
                    ck = i // _CH
                    j = i % _CH
                    n_k = min(_CH, b_sw - ck * _CH)
                    posmap[(s, k)] = (pos + ck * _CH
                                      + (j % 128) * (n_k // 128) + j // 128)
                pos += b_sw
        # gidx stream per phase-A chunk (local window idx, pad=0)
        for (s, w, pos, n) in chunksA:
            u = uniq[(c, s, w)]
            off = pos - wpos[(s, w)]
            seg = (u[off:off + n] - w * _WIN).astype(np.int16)
            gidx_parts.append(_pack16(seg, n, np.int16(0)))
        gidx = np.concatenate(gidx_parts, axis=1)

        # ridx stream per phase-B chunk (position within superblock region)
        ridx_parts = []
        for (s, p, n) in chunksB:
            sk = slot_key[c, p:p + n]
            loc = np.zeros(n, dtype=np.int16)
            nzm = sk >= 0
            if nzm.any():
                loc[nzm] = np.array(
                    [posmap[(s, k)] for k in sk[nzm]],
                    dtype=np.int64) - sb_base[s]
            ridx_parts.append(_pack16(loc, n, np.int16(0)))
        ridx = np.concatenate(ridx_parts, axis=1)

        # A^T one-hot [128, T*BLK] bf16  (slot -> row-in-block)
        atab = np.zeros((128, T * _BLK), dtype=np.float32)
        sl = np.arange(nslot)
        colv = sl // 128
        pv = sl % 128
        realm = slot_key[c] >= 0
        blkv = np.zeros(nslot, dtype=np.int64)
        blkv[realm] = slot_row[c][realm] // _BLK
        atab[pv[realm], colv[realm] * _BLK
             + (slot_row[c][realm] - blkv[realm] * _BLK)] = 1.0

        keys, rows = cores[c]
        counts = np.bincount(rows, minlength=_RC).astype(np.float32)
        invc = 1.0 / np.maximum(counts, 1.0)

        in_maps.append({
            "gidx": np.ascontiguousarray(gidx),
            "ridx": np.ascontiguousarray(ridx),
            "atab": atab.astype(_BF16),
            "invc": np.broadcast_to(invc.astype(_BF16), (64, _RC)).copy(),
        })
    struct = (tuple(int(x) for x in tiles_pb), tuple(chunksA),
              tuple(chunksB), spad, tuple(int(x) for x in sb_base))
    return struct, in_maps


def _build(struct, n_reps=1):
    from concourse import bacc, mybir, tile

    tiles_pb, chunksA, chunksB, spad, sb_base = struct
    T = sum(tiles_pb)
    glen = sum(n for (_s, _w, _p, n) in chunksA)
    rlen = sum(n for (_s, _p, n) in chunksB)

    nc = bacc.Bacc(None, target_bir_lowering=False, debug=False,
                   num_swdge_queues=1)
    table = nc.dram_tensor("table", [_V, _D], mybir.dt.float32,
                           kind="ExternalInput")
    gidx_d = nc.dram_tensor("gidx", [128, glen // 16], mybir.dt.int16,
                            kind="ExternalInput")
    ridx_d = nc.dram_tensor("ridx", [128, rlen // 16], mybir.dt.int16,
                            kind="ExternalInput")
    atab_d = nc.dram_tensor("atab", [128, T * _BLK], mybir.dt.bfloat16,
                            kind="ExternalInput")
    invc_d = nc.dram_tensor("invc", [64, _RC], mybir.dt.bfloat16,
                            kind="ExternalInput")
    out = nc.dram_tensor("out", [64, _RC], mybir.dt.float32,
                         kind="ExternalOutput")

    with tile.TileContext(nc) as tc:
        with (
            tc.tile_pool(name="meta", bufs=1) as mpool,
            tc.tile_pool(name="stg", bufs=3, space="DRAM") as spool,
            tc.tile_pool(name="ga", bufs=6) as gapool,
            tc.tile_pool(name="gb", bufs=4) as gbpool,
            tc.tile_pool(name="gc", bufs=3) as gcpool,
            tc.tile_pool(name="ps", bufs=2, space="PSUM") as ppool,
        ):
            gidx_sb = mpool.tile([128, glen // 16], mybir.dt.int16, tag="gi")
            ridx_sb = mpool.tile([128, rlen // 16], mybir.dt.int16, tag="ri")
            atab_sb = mpool.tile([128, T, _BLK], mybir.dt.bfloat16, tag="at")
            invc_sb = mpool.tile([64, _RC], mybir.dt.bfloat16, tag="iv")
            outT = mpool.tile([64, _RC], mybir.dt.bfloat16, tag="oT")
            nc.sync.dma_start(out=gidx_sb[:], in_=gidx_d[:])
            nc.sync.dma_start(out=ridx_sb[:], in_=ridx_d[:])
            nc.sync.dma_start(
                out=atab_sb[:],
                in_=atab_d[:].rearrange("p (t m) -> p t m", m=_BLK))
            nc.sync.dma_start(out=invc_sb[:], in_=invc_d[:])

            for _i in range(6):
                gz = gapool.tile([128, (_CH // 128) * _D], mybir.dt.float32,
                                 tag="ga")
                nc.vector.memset(gz[:], 0.0)
            for _i in range(4):
                gz2 = gbpool.tile([128, (_CHB // 128) * _D], mybir.dt.float32,
                                  tag="gb")
                nc.vector.memset(gz2[:], 0.0)

            for _rep in range(n_reps):
                stage = spool.tile([spad, _D], mybir.dt.float32, tag="st")
                # ---- phase A: window gather -> staging ----
                goff = 0
                wr = 0
                for (s, w, pos, n) in chunksA:
                    base = w * _WIN
                    wsize = min(_WIN, _V - base)
                    cc = n // 128
                    ga = gapool.tile([128, (_CH // 128) * _D],
                                     mybir.dt.float32, tag="ga")
                    nc.gpsimd.dma_gather(
                        out_ap=ga[:, :cc * _D].rearrange(
                            "p (cc d) -> p cc d", d=_D),
                        in_ap=table[base:base + wsize, :],
                        idxs_ap=gidx_sb[:, goff // 16:(goff + n) // 16],
                        num_idxs=n, num_idxs_reg=n,
                        elem_size=_D, queue_num=0,
                    )
                    # transposed-within-chunk staging: partition p holds
                    # staging rows [pos + p*cc, pos + (p+1)*cc) contiguously
                    dst = stage[pos:pos + n, :].rearrange(
                        "(p cc) d -> p (cc d)", p=128)
                    eng = nc.sync if wr % 8 < 5 else nc.scalar
                    eng.dma_start(out=dst, in_=ga[:, :cc * _D])
                    wr += 1
                    goff += n

                # ---- phase B: row-major gather + matmul combine ----
                roff = 0
                bi = 0          # index into chunksB
                gcur = None
                gbf = None
                gend = 0
                gbase = 0
                col = 0
                for g in range(_NGRP):
                    psum = ppool.tile([64, _BANK], mybir.dt.float32, tag="ps")
                    for bb in range(_BPG):
                        b = g * _BPG + bb
                        ntile = int(tiles_pb[b])
                        for t in range(ntile):
                            if col * 128 >= gend:
                                (s, p, n) = chunksB[bi]
                                bi += 1
                                gbase = p
                                gend = p + n
                                sb0 = sb_base[s]
                                sbn = min(sb_base[s + 1], spad) - sb0
                                gcur = gbpool.tile([128, (_CHB // 128) * _D],
                                                   mybir.dt.float32, tag="gb")
                                nc.gpsimd.dma_gather(
                                    out_ap=gcur[:, :_cdiv(n, 128) * _D]
                                    .rearrange("p (cc d) -> p cc d", d=_D),
                                    in_ap=stage[sb0:sb0 + sbn, :],
                                    idxs_ap=ridx_sb[:, roff // 16:
                                                    (roff + n) // 16],
                                    num_idxs=n, num_idxs_reg=n,
                                    elem_size=_D, queue_num=0,
                                )
                                roff += n
                                gbf = gcpool.tile([128, (_CHB // 128) * _D],
                                                  mybir.dt.bfloat16, tag="gc")
                                if bi % 2 == 0:
                                    nc.vector.tensor_copy(
                                        out=gbf[:], in_=gcur[:])
                                else:
                                    nc.scalar.copy(
                                        out=gbf[:], in_=gcur[:])
                            lc = col - gbase // 128
                            nc.tensor.matmul(
                                out=psum[:, bb * _BLK:(bb + 1) * _BLK],
                                lhsT=gbf[:, lc * _D:(lc + 1) * _D],
                                rhs=atab_sb[:, col, :],
                                start=(t == 0),
                                stop=(t == ntile - 1),
                            )
                            col += 1
                    nc.vector.tensor_tensor(
                        out=outT[:, g * _BANK:(g + 1) * _BANK],
                        in0=psum[:],
                        in1=invc_sb[:, g * _BANK:(g + 1) * _BANK],
                        op=mybir.AluOpType.mult,
                    )
            nc.gpsimd.dma_start(out=out[:], in_=outT[:])
    nc.compile()
    return nc


def _state(values, row_indices, emb_table, n_reps=1):
    struct, in_maps = _prep(values, row_indices)
    key = (struct, n_reps)
    if key not in _prog_cache:
        _prog_cache[key] = _build(struct, n_reps=n_reps)
    nc = _prog_cache[key]
    table = np.ascontiguousarray(np.asarray(emb_table, dtype=np.float32))
    for m in in_maps:
        m["table"] = table
    return nc, in_maps


def kernel(values, row_indices, emb_table):
    from concourse.bass_utils import run_bass_kernel_spmd

    nc, in_maps = _state(values, row_indices, emb_table)
    res = run_bass_kernel_spmd(nc, in_maps, core_ids=list(range(_M)))
    full = np.concatenate(
        [np.asarray(res.results[c]["out"]).T for c in range(_M)], axis=0)
    return np.ascontiguousarray(full.reshape(_B, _S, _D).astype(np.float32))
